# revision 1
# baseline (speedup 1.0000x reference)
"""CoAttention forward on 8 TRN2 NeuronCores.

Data-parallel over batch B=64 (8 batches/core). Heavy matmuls run as f16
hi/lo multi-pass on the PE (3-pass ~22-bit mantissa, 1 cyc/row vs fp32's
4 cyc/row); the logit path (H storage, w_h dots, softmax) stays fp32.

Per batch b (Q [512,1024], V [196,1024], D=1024):
  A    = W_b^T Q^T               [D, NQ]   3-pass f16, split hi/lo on chip
  C    = tanh(A^T V^T)           [NQ, NV]  3-pass, stored f16
  CT   = C^T                     via PE f16 transposes
  WqQT = Q W_q^T                 [NQ, D]   3-pass, split hi/lo
  WvVT = V W_v^T                 [NV, D]   3-pass, split hi/lo
  G_v  = transpose(WvVT_hi) + (WqQT_hi + WqQT_lo) C     (f16 psum + f32 psum, DVE add)
  H_v  = tanh(G_v)  fp32;  h_v = w_hv^T H_v  fp32 -> softmax -> a_v
  G_q  = transpose(WqQT_hi) + (WvVT_hi + WvVT_lo) CT
  H_q  = tanh(G_q)  fp32;  h_q -> softmax -> a_q
  v_hat = sum_v a_v[v] VT_hi[:, v];  q_hat = sum_q a_q[q] QT_hi[:, q]

kernel(**inputs) takes FULL inputs, shards internally, returns (v_hat, q_hat).
"""
import numpy as np

import concourse.bass as bass
import concourse.mybir as mybir
import concourse.tile as tile
from concourse import bacc
from concourse.bass_utils import run_bass_kernel_spmd
from concourse.masks import make_identity

AF = mybir.ActivationFunctionType
ALU = mybir.AluOpType
AX = mybir.AxisListType
F32 = mybir.dt.float32
F16 = mybir.dt.float16

B, NV, NQ, D = 64, 196, 512, 1024
NCORES = 8
NB = B // NCORES          # batches per core
KD = D // 128             # 8 feature k-tiles
MQ = NQ // 128            # 4 NQ m-tiles
NV1 = NV - 128            # 68 (second NV tile)


def build(nb=NB):
    nc = bacc.Bacc(None, target_bir_lowering=False)

    QTh_d = nc.dram_tensor("QTh", [nb, D, NQ], F16, kind="ExternalInput")
    QTl_d = nc.dram_tensor("QTl", [nb, D, NQ], F16, kind="ExternalInput")
    VTh_d = nc.dram_tensor("VTh", [nb, D, NV], F16, kind="ExternalInput")
    VTl_d = nc.dram_tensor("VTl", [nb, D, NV], F16, kind="ExternalInput")
    Wbh_d = nc.dram_tensor("Wbh", [D, D], F16, kind="ExternalInput")
    Wbl_d = nc.dram_tensor("Wbl", [D, D], F16, kind="ExternalInput")
    WqTh_d = nc.dram_tensor("WqTh", [D, D], F16, kind="ExternalInput")
    WqTl_d = nc.dram_tensor("WqTl", [D, D], F16, kind="ExternalInput")
    WvTh_d = nc.dram_tensor("WvTh", [D, D], F16, kind="ExternalInput")
    WvTl_d = nc.dram_tensor("WvTl", [D, D], F16, kind="ExternalInput")
    whv_d = nc.dram_tensor("whv", [D, 1], F32, kind="ExternalInput")
    whq_d = nc.dram_tensor("whq", [D, 1], F32, kind="ExternalInput")
    OV_d = nc.dram_tensor("OV", [nb, D], F32, kind="ExternalOutput")
    OQ_d = nc.dram_tensor("OQ", [nb, D], F32, kind="ExternalOutput")

    with tile.TileContext(nc) as tc:
        with (
            tc.tile_pool(name="wsb", bufs=1) as wsb,
            tc.tile_pool(name="iop", bufs=2) as iop,
            tc.tile_pool(name="mid", bufs=1) as mid,
            tc.tile_pool(name="sm", bufs=1) as sm,
            tc.tile_pool(name="psp", bufs=4, space="PSUM") as psp,
        ):
            # ---- persistent weights ----
            def wtile(name, src):
                t = wsb.tile([128, KD, D], F16, name=name)
                nc.sync.dma_start(out=t, in_=src.rearrange("(k p) d -> p k d", p=128))
                return t

            wbh = wtile("wbh", Wbh_d)
            wbl = wtile("wbl", Wbl_d)
            wqth = wtile("wqth", WqTh_d)
            wqtl = wtile("wqtl", WqTl_d)
            wvth = wtile("wvth", WvTh_d)
            wvtl = wtile("wvtl", WvTl_d)
            whv_sb = wsb.tile([128, KD], F32)
            nc.sync.dma_start(out=whv_sb, in_=whv_d[:, 0].rearrange("(k p) -> p k", p=128))
            whq_sb = wsb.tile([128, KD], F32)
            nc.sync.dma_start(out=whq_sb, in_=whq_d[:, 0].rearrange("(k p) -> p k", p=128))
            identh = wsb.tile([128, 128], F16)
            make_identity(nc, identh)
            ones_row = wsb.tile([1, 128], F32)
            nc.vector.memset(ones_row, 1.0)

            for b in range(nb):
                qth = iop.tile([128, KD, NQ], F16, tag="qth")
                nc.sync.dma_start(out=qth, in_=QTh_d[b].rearrange("(k p) n -> p k n", p=128))
                qtl = iop.tile([128, KD, NQ], F16, tag="qtl")
                nc.sync.dma_start(out=qtl, in_=QTl_d[b].rearrange("(k p) n -> p k n", p=128))
                vth = iop.tile([128, KD, NV], F16, tag="vth")
                nc.sync.dma_start(out=vth, in_=VTh_d[b].rearrange("(k p) n -> p k n", p=128))
                vtl = iop.tile([128, KD, NV], F16, tag="vtl")
                nc.sync.dma_start(out=vtl, in_=VTl_d[b].rearrange("(k p) n -> p k n", p=128))

                # ---- phase 1+2 interleaved: A (3-pass), then C (3-pass) ----
                a_hi = mid.tile([128, KD, NQ], F16, tag="a_hi")
                a_lo = mid.tile([128, KD, NQ], F16, tag="a_lo")
                c_ps = [psp.tile([128, NV], F32, tag="ps196", name=f"c_ps{b}_{m}")
                        for m in range(MQ)]

                def emit_a(e):
                    pa = psp.tile([128, NQ], F32, tag="ps512", bufs=4, name=f"pa{b}_{e}")
                    passes = ((wbh, qth), (wbh, qtl), (wbl, qth))
                    es = slice(e * 128, (e + 1) * 128)
                    n = 0
                    for k in range(KD):
                        for lh, rh in passes:
                            n += 1
                            nc.tensor.matmul(pa, lh[:, k, es], rh[:, k, :],
                                             start=(n == 1), stop=(n == 3 * KD))
                    nc.vector.tensor_copy(a_hi[:, e, :], pa)
                    nc.vector.tensor_sub(a_lo[:, e, :], pa, a_hi[:, e, :])

                def emit_c(e):
                    for m in range(MQ):
                        ms = slice(m * 128, (m + 1) * 128)
                        for i, (lh, rh) in enumerate(((a_hi, vth), (a_hi, vtl), (a_lo, vth))):
                            nc.tensor.matmul(c_ps[m], lh[:, e, ms], rh[:, e, :],
                                             start=(e == 0 and i == 0),
                                             stop=(e == KD - 1 and i == 2))

                for e in range(KD + 1):
                    if e < KD:
                        emit_a(e)
                    if e >= 1:
                        emit_c(e - 1)

                c_sb = mid.tile([128, MQ, NV], F16, tag="c")
                for m in range(MQ):
                    nc.scalar.activation(c_sb[:, m, :], c_ps[m], AF.Tanh)

                # ---- CT via f16 PE transposes of C ----
                ct_sb = mid.tile([128, 2, NQ], F16, tag="ct")
                for mv in range(2):
                    rows = 128 if mv == 0 else NV1
                    ctp = psp.tile([128, NQ], F16, tag="ps512", bufs=4, name=f"ctp{b}_{mv}")
                    for mq in range(MQ):
                        nc.tensor.matmul(
                            ctp[:rows, mq * 128:(mq + 1) * 128],
                            c_sb[:, mq, mv * 128:mv * 128 + rows],
                            identh, is_transpose=True,
                            start=(mq == 0), stop=(mq == MQ - 1))
                    nc.scalar.copy(ct_sb[:rows, mv, :], ctp[:rows, :])

                # ---- phase 3: WqQT, WvVT (3-pass, split hi/lo) ----
                wqqt_hi = mid.tile([128, MQ, D], F16, tag="wqqt_hi")
                wqqt_lo = mid.tile([128, MQ, D], F16, tag="wqqt_lo")
                for m in range(MQ):
                    ms = slice(m * 128, (m + 1) * 128)
                    for h in range(2):
                        hs = slice(h * 512, (h + 1) * 512)
                        p = psp.tile([128, 512], F32, tag="ps512", bufs=4, name=f"pq{b}_{m}_{h}")
                        n = 0
                        for k in range(KD):
                            for lh, rh in ((qth, wqth), (qth, wqtl), (qtl, wqth)):
                                n += 1
                                nc.tensor.matmul(p, lh[:, k, ms], rh[:, k, hs],
                                                 start=(n == 1), stop=(n == 3 * KD))
                        nc.vector.tensor_copy(wqqt_hi[:, m, hs], p)
                        nc.vector.tensor_sub(wqqt_lo[:, m, hs], p, wqqt_hi[:, m, hs])
                wvvt_hi = mid.tile([128, 2, D], F16, tag="wvvt_hi")
                wvvt_lo = mid.tile([128, 2, D], F16, tag="wvvt_lo")
                for m in range(2):
                    rows = 128 if m == 0 else NV1
                    ms = slice(m * 128, m * 128 + rows)
                    for h in range(2):
                        hs = slice(h * 512, (h + 1) * 512)
                        p = psp.tile([128, 512], F32, tag="ps512", bufs=4, name=f"pv{b}_{m}_{h}")
                        n = 0
                        for k in range(KD):
                            for lh, rh in ((vth, wvth), (vth, wvtl), (vtl, wvth)):
                                n += 1
                                nc.tensor.matmul(p[:rows, :], lh[:, k, ms], rh[:, k, hs],
                                                 start=(n == 1), stop=(n == 3 * KD))
                        nc.vector.tensor_copy(wvvt_hi[:rows, m, hs], p[:rows, :])
                        nc.vector.tensor_sub(wvvt_lo[:rows, m, hs], p[:rows, :],
                                             wvvt_hi[:rows, m, hs])

                # ---- phase 4: H_v (f32) + h_v ----
                hv_m_l = [None] * KD
                h_v_ps = psp.tile([1, NV], F32, tag="ps196", name=f"hv_acc{b}")

                def emit_hv(m):
                    ms = slice(m * 128, (m + 1) * 128)
                    t2 = psp.tile([128, NV], F32, tag="ps196", name=f"hv2_{b}_{m}")
                    for kq in range(MQ):
                        for i, lh in enumerate((wqqt_hi, wqqt_lo)):
                            nc.tensor.matmul(t2, lh[:, kq, ms], c_sb[:, kq, :],
                                             start=(kq == 0 and i == 0),
                                             stop=(kq == MQ - 1 and i == 1))
                    t1sb = [None, None]
                    for i, w in enumerate((wvvt_hi, wvvt_lo)):
                        t1 = psp.tile([128, NV], F16, tag="ps196",
                                      name=f"hv1_{b}_{m}_{i}")
                        nc.tensor.matmul(t1[:, 0:128], w[:, 0, ms], identh,
                                         is_transpose=True, start=True, stop=False)
                        nc.tensor.matmul(t1[:, 128:NV], w[:NV1, 1, ms],
                                         identh[:NV1, :NV1],
                                         is_transpose=True, start=False, stop=True)
                        t1sb[i] = sm.tile([128, NV], F16, tag=f"t1v{i}", bufs=2,
                                          name=f"t1v{b}_{m}_{i}")
                        nc.scalar.copy(t1sb[i], t1)
                    pre = sm.tile([128, NV], F32, tag="prev", bufs=1, name=f"prev{b}_{m}")
                    nc.vector.scalar_tensor_tensor(out=pre, in0=t2, scalar=1.0, in1=t1sb[0],
                                                   op0=ALU.mult, op1=ALU.add)
                    nc.vector.tensor_add(pre, pre, t1sb[1])
                    hv_m = sm.tile([128, NV], F32, tag="hvm", bufs=2, name=f"hvm{b}_{m}")
                    nc.scalar.activation(hv_m, pre, AF.Tanh)
                    hv_m_l[m] = hv_m

                def emit_hv_dot(m):
                    nc.tensor.matmul(h_v_ps, whv_sb[:, m:m + 1], hv_m_l[m],
                                     start=(m == 0), stop=(m == KD - 1))

                for m in range(KD + 1):
                    if m < KD:
                        emit_hv(m)
                    if m >= 1:
                        emit_hv_dot(m - 1)

                # ---- phase 5: H_q (f32) + h_q ----
                hq_m_l = [None] * KD
                h_q_ps = psp.tile([1, NQ], F32, tag="ps512", bufs=4, name=f"hq_acc{b}")

                def emit_hq(m):
                    ms = slice(m * 128, (m + 1) * 128)
                    t2 = psp.tile([128, NQ], F32, tag="ps512", bufs=4, name=f"hq2_{b}_{m}")
                    for kv in range(2):
                        rows = 128 if kv == 0 else NV1
                        for i, lh in enumerate((wvvt_hi, wvvt_lo)):
                            nc.tensor.matmul(t2, lh[:rows, kv, ms], ct_sb[:rows, kv, :],
                                             start=(kv == 0 and i == 0),
                                             stop=(kv == 1 and i == 1))
                    t1sb = [None, None]
                    for i, w in enumerate((wqqt_hi, wqqt_lo)):
                        t1 = psp.tile([128, NQ], F16, tag="ps512", bufs=4,
                                      name=f"hq1_{b}_{m}_{i}")
                        for kq in range(MQ):
                            nc.tensor.matmul(t1[:, kq * 128:(kq + 1) * 128],
                                             w[:, kq, ms], identh, is_transpose=True,
                                             start=(kq == 0), stop=(kq == MQ - 1))
                        t1sb[i] = sm.tile([128, NQ], F16, tag=f"t1q{i}", bufs=2,
                                          name=f"t1q{b}_{m}_{i}")
                        nc.scalar.copy(t1sb[i], t1)
                    pre = sm.tile([128, NQ], F32, tag="preq", bufs=1, name=f"preq{b}_{m}")
                    nc.vector.scalar_tensor_tensor(out=pre, in0=t2, scalar=1.0, in1=t1sb[0],
                                                   op0=ALU.mult, op1=ALU.add)
                    nc.vector.tensor_add(pre, pre, t1sb[1])
                    hq_m = sm.tile([128, NQ], F32, tag="hqm", bufs=2, name=f"hqm{b}_{m}")
                    nc.scalar.activation(hq_m, pre, AF.Tanh)
                    hq_m_l[m] = hq_m

                def emit_hq_dot(m):
                    nc.tensor.matmul(h_q_ps, whq_sb[:, m:m + 1], hq_m_l[m],
                                     start=(m == 0), stop=(m == KD - 1))

                for m in range(KD + 1):
                    if m < KD:
                        emit_hq(m)
                    if m >= 1:
                        emit_hq_dot(m - 1)

                # ---- phase 6: softmaxes + on-chip broadcast ----
                def softmax_bcast(h_ps, n, tagp):
                    negm = sm.tile([1, 1], F32, tag=f"negm{tagp}")
                    nc.vector.reduce_max(negm, h_ps, axis=AX.X, negate=True)
                    ex = sm.tile([1, n], F32, tag=f"ex{tagp}")
                    ssum = sm.tile([1, 1], F32, tag=f"ssum{tagp}")
                    nc.scalar.activation(ex, h_ps, AF.Exp, bias=negm, accum_out=ssum)
                    rs = sm.tile([1, 1], F32, tag=f"rs{tagp}")
                    nc.vector.reciprocal(rs, ssum)
                    ones_s = sm.tile([1, 128], F32, tag=f"ones_s{tagp}")
                    nc.vector.tensor_scalar_mul(ones_s, ones_row, rs)
                    ab_ps = psp.tile([128, n], F32, tag="ps512", bufs=4, name=f"abps{tagp}{b}")
                    nc.tensor.matmul(ab_ps, ones_s, ex, start=True, stop=True)
                    ab = sm.tile([128, n], F32, tag=f"ab{tagp}")
                    nc.scalar.copy(ab, ab_ps)
                    return ab

                av_b = softmax_bcast(h_v_ps, NV, "v")
                aq_b = softmax_bcast(h_q_ps, NQ, "q")

                # ---- phase 7: v_hat / q_hat (hi+lo two-pass) ----
                vhat_sb = sm.tile([128, KD], F32, tag="vhat")
                vhat2_sb = sm.tile([128, KD], F32, tag="vhat2")
                scrv = sm.tile([128, NV], F16, tag="scrv")
                for k in range(KD):
                    nc.vector.scalar_tensor_tensor(
                        out=scrv, in0=vth[:, k, :], scalar=1.0, in1=av_b,
                        op0=ALU.mult, op1=ALU.mult, accum_out=vhat_sb[:, k:k + 1])
                    nc.vector.scalar_tensor_tensor(
                        out=scrv, in0=vtl[:, k, :], scalar=1.0, in1=av_b,
                        op0=ALU.mult, op1=ALU.mult, accum_out=vhat2_sb[:, k:k + 1])
                nc.vector.tensor_add(vhat_sb, vhat_sb, vhat2_sb)
                qhat_sb = sm.tile([128, KD], F32, tag="qhat")
                qhat2_sb = sm.tile([128, KD], F32, tag="qhat2")
                scrq = sm.tile([128, NQ], F16, tag="scrq")
                for k in range(KD):
                    nc.vector.scalar_tensor_tensor(
                        out=scrq, in0=qth[:, k, :], scalar=1.0, in1=aq_b,
                        op0=ALU.mult, op1=ALU.mult, accum_out=qhat_sb[:, k:k + 1])
                    nc.vector.scalar_tensor_tensor(
                        out=scrq, in0=qtl[:, k, :], scalar=1.0, in1=aq_b,
                        op0=ALU.mult, op1=ALU.mult, accum_out=qhat2_sb[:, k:k + 1])
                nc.vector.tensor_add(qhat_sb, qhat_sb, qhat2_sb)
                nc.sync.dma_start(out=OV_d[b].rearrange("(k p) -> p k", p=128), in_=vhat_sb)
                nc.sync.dma_start(out=OQ_d[b].rearrange("(k p) -> p k", p=128), in_=qhat_sb)

    nc.finalize()
    return nc


_BUILT = {}


def _split(x):
    hi = x.astype(np.float16)
    lo = (x - hi.astype(np.float32)).astype(np.float16)
    return np.ascontiguousarray(hi), np.ascontiguousarray(lo)


def kernel(V, Q, W_b, W_v, W_q, w_hv, w_hq, _trace=False):
    V = np.asarray(V, dtype=np.float32)
    Q = np.asarray(Q, dtype=np.float32)
    nb = B // NCORES
    QTh, QTl = _split(Q.transpose(0, 2, 1))      # [B, D, NQ] f16
    VTh, VTl = _split(V.transpose(0, 2, 1))      # [B, D, NV] f16
    Wbh, Wbl = _split(np.asarray(W_b, dtype=np.float32))
    WqTh, WqTl = _split(np.asarray(W_q, dtype=np.float32).T)
    WvTh, WvTl = _split(np.asarray(W_v, dtype=np.float32).T)
    whv = np.ascontiguousarray(np.asarray(w_hv, dtype=np.float32))
    whq = np.ascontiguousarray(np.asarray(w_hq, dtype=np.float32))

    if nb not in _BUILT:
        _BUILT[nb] = build(nb)
    nc = _BUILT[nb]

    in_maps = []
    for c in range(NCORES):
        sl = slice(c * nb, (c + 1) * nb)
        in_maps.append({
            "QTh": np.ascontiguousarray(QTh[sl]), "QTl": np.ascontiguousarray(QTl[sl]),
            "VTh": np.ascontiguousarray(VTh[sl]), "VTl": np.ascontiguousarray(VTl[sl]),
            "Wbh": Wbh, "Wbl": Wbl, "WqTh": WqTh, "WqTl": WqTl,
            "WvTh": WvTh, "WvTl": WvTl, "whv": whv, "whq": whq,
        })

    out = run_bass_kernel_spmd(nc, in_maps, core_ids=list(range(NCORES)),
                               trace=_trace)
    v_hat = np.concatenate([out.results[c]["OV"] for c in range(NCORES)], axis=0)
    q_hat = np.concatenate([out.results[c]["OQ"] for c in range(NCORES)], axis=0)
    if _trace:
        kernel._last_exec_ns = out.exec_time_ns
        kernel._last_results = out
    return (v_hat, q_hat)



# revision 2
# speedup vs baseline: 1.0028x; 1.0028x over previous
"""CoAttention forward on 8 TRN2 NeuronCores — fp8-corrected restructured version.

Data-parallel over batch B=64 (8 batches/core). Key structure per batch b
(Q [512,1024], V [196,1024], D=1024):

  A'   = Wb @ V^T            [D, NV]   3-pass f16 (hi/lo split on chip)
  C    = tanh(Q @ A')        [NQ, NV]  3-pass f16; stored f16 hi + f16 lo
  WqQT = Q @ Wq^T            [NQ, D]   psum x4096: f16 main + fp8 DoubleRow corr
  WvVT = V @ Wv^T            [NV, D]   same
  H_v  = tanh(WqQT^T C + WvVT^T)   one psum x256: f16 main + fp8 DR corr
                                    + identity-matmul transpose additive
  H_q  = tanh(WvVT^T C^T + WqQT^T) same
  h dots in f16, softmax with on-chip PE broadcast, phase7 DVE weighted sums.

kernel(**inputs) takes FULL inputs, shards internally, returns (v_hat, q_hat).
"""
import numpy as np
import ml_dtypes

import concourse.bass as bass
import concourse.mybir as mybir
import concourse.tile as tile
from concourse import bacc
from concourse.bass_utils import run_bass_kernel_spmd
from concourse.masks import make_identity

AF = mybir.ActivationFunctionType
ALU = mybir.AluOpType
AX = mybir.AxisListType
PM = mybir.MatmulPerfMode
F32 = mybir.dt.float32
F16 = mybir.dt.float16
F8 = mybir.dt.float8e4

B, NV, NQ, D = 64, 196, 512, 1024
NCORES = 8
NB = B // NCORES          # batches per core
KD = D // 128             # 8 feature k-tiles
MQ = NQ // 128            # 4 NQ m-tiles
NV1 = NV - 128            # 68 (second NV tile)
NVP = 208                 # NV padded to /16 for fp8 DoubleRow k-tile stride

S_W = 4096.0              # weight-GEMM psum scale
S_H = 256.0               # H-path psum scale (hiS tensors are 256*value)


def build(nb=NB):
    nc = bacc.Bacc(None, target_bir_lowering=False)

    QTh_d = nc.dram_tensor("QTh", [nb, D, NQ], F16, kind="ExternalInput")
    QTl_d = nc.dram_tensor("QTl", [nb, D, NQ], F16, kind="ExternalInput")
    VTh_d = nc.dram_tensor("VTh", [nb, D, NV], F16, kind="ExternalInput")
    VTl_d = nc.dram_tensor("VTl", [nb, D, NV], F16, kind="ExternalInput")
    WbTh_d = nc.dram_tensor("WbTh", [D, D], F16, kind="ExternalInput")
    WbTl_d = nc.dram_tensor("WbTl", [D, D], F16, kind="ExternalInput")
    WqTS_d = nc.dram_tensor("WqTS", [D, D], F16, kind="ExternalInput")   # Wq^T*4096
    WqTh8_d = nc.dram_tensor("WqTh8", [D, D], F8, kind="ExternalInput")  # fp8(Wq^T)
    WqTl8_d = nc.dram_tensor("WqTl8", [D, D], F8, kind="ExternalInput")  # fp8(lo*4096)
    WvTS_d = nc.dram_tensor("WvTS", [D, D], F16, kind="ExternalInput")
    WvTh8_d = nc.dram_tensor("WvTh8", [D, D], F8, kind="ExternalInput")
    WvTl8_d = nc.dram_tensor("WvTl8", [D, D], F8, kind="ExternalInput")
    whv_d = nc.dram_tensor("whv", [D, 1], F16, kind="ExternalInput")
    whq_d = nc.dram_tensor("whq", [D, 1], F16, kind="ExternalInput")
    OV_d = nc.dram_tensor("OV", [nb, D], F32, kind="ExternalOutput")
    OQ_d = nc.dram_tensor("OQ", [nb, D], F32, kind="ExternalOutput")

    with tile.TileContext(nc) as tc:
        with (
            tc.tile_pool(name="wsb", bufs=1) as wsb,
            tc.tile_pool(name="iop", bufs=2) as iop,
            tc.tile_pool(name="io8", bufs=1) as io8,
            tc.tile_pool(name="mid", bufs=1) as mid,
            tc.tile_pool(name="sm", bufs=1) as sm,
            tc.tile_pool(name="psp", bufs=6, space="PSUM") as psp,
        ):
            # ---- persistent weights (per-k chunked DMAs for fine-grained deps
            # so the first A'/C/WqQT matmuls start as soon as their chunk lands) ----
            def wtile(name, src, dt=F16):
                t = wsb.tile([128, KD, D], dt, name=name)
                r = src.rearrange("(k p) d -> p k d", p=128)
                for k in range(KD):
                    nc.sync.dma_start(out=t[:, k, :], in_=r[:, k, :])
                return t

            wbth = wtile("wbth", WbTh_d)
            wbtl = wtile("wbtl", WbTl_d)

            def load_inputs(b):
                vth = iop.tile([128, KD, NV], F16, tag="vth", name=f"vth{b}")
                nc.sync.dma_start(out=vth, in_=VTh_d[b].rearrange("(k p) n -> p k n", p=128))
                vtl = iop.tile([128, KD, NV], F16, tag="vtl", name=f"vtl{b}")
                nc.sync.dma_start(out=vtl, in_=VTl_d[b].rearrange("(k p) n -> p k n", p=128))
                qth = iop.tile([128, KD, NQ], F16, tag="qth", name=f"qth{b}")
                nc.sync.dma_start(out=qth, in_=QTh_d[b].rearrange("(k p) n -> p k n", p=128))
                qtl = iop.tile([128, KD, NQ], F16, tag="qtl", name=f"qtl{b}")
                nc.sync.dma_start(out=qtl, in_=QTl_d[b].rearrange("(k p) n -> p k n", p=128))
                qh8 = io8.tile([128, KD, NQ], F8, tag="qh8", name=f"qh8{b}")
                nc.vector.tensor_copy(qh8, qth)
                ql8 = io8.tile([128, KD, NQ], F8, tag="ql8", name=f"ql8{b}")
                nc.vector.tensor_scalar_mul(ql8, qtl, S_W)
                vh8 = io8.tile([128, KD, NVP], F8, tag="vh8", name=f"vh8{b}")
                nc.vector.tensor_copy(vh8[:, :, 0:NV], vth)
                vl8 = io8.tile([128, KD, NVP], F8, tag="vl8", name=f"vl8{b}")
                nc.vector.tensor_scalar_mul(vl8[:, :, 0:NV], vtl, S_W)
                return qth, qtl, vth, vtl, qh8, ql8, vh8, vl8

            ins0 = load_inputs(0)

            wqts = wtile("wqts", WqTS_d)
            wqth8 = wtile("wqth8", WqTh8_d, F8)
            wqtl8 = wtile("wqtl8", WqTl8_d, F8)
            wvts = wtile("wvts", WvTS_d)
            wvth8 = wtile("wvth8", WvTh8_d, F8)
            wvtl8 = wtile("wvtl8", WvTl8_d, F8)
            whv_sb = wsb.tile([128, KD], F16)
            nc.sync.dma_start(out=whv_sb, in_=whv_d[:, 0].rearrange("(k p) -> p k", p=128))
            whq_sb = wsb.tile([128, KD], F16)
            nc.sync.dma_start(out=whq_sb, in_=whq_d[:, 0].rearrange("(k p) -> p k", p=128))
            identh = wsb.tile([128, 128], F16)
            make_identity(nc, identh)
            ones_row = wsb.tile([1, 128], F32)
            nc.vector.memset(ones_row, 1.0)

            # per-batch tensors with static zero rows for fp8 DR padding:
            # allocated once; rows [NV1:128, 1, :] memset once, never rewritten.
            ct_hi = wsb.tile([128, 2, NQ], F16)
            ct_h8 = wsb.tile([128, 2, NQ], F8)
            ct_l8 = wsb.tile([128, 2, NQ], F8)
            wvvt_hiS = wsb.tile([128, 2, D], F16)
            wvvt_hi8 = wsb.tile([128, 2, D], F8)
            wvvt_lo8 = wsb.tile([128, 2, D], F8)
            # engine base partition must be 0/32/64/96: zero [64:128] once; the
            # per-batch writes re-fill rows [64:NV1] so only [NV1:128] stay 0.
            for t in (ct_h8, ct_l8, wvvt_hi8, wvvt_lo8):
                nc.vector.memset(t[64:128, 1, :], 0.0)

            emit_tail_prev = None
            for b in range(nb):
                qth, qtl, vth, vtl, qh8, ql8, vh8, vl8 = (
                    ins0 if b == 0 else load_inputs(b))

                # ---- phase 1: A' (3-pass); phase 2: C (3-pass, per q-tile) ----
                a_hi = mid.tile([128, KD, NV], F16, tag="a_hi")
                a_lo = mid.tile([128, KD, NV], F16, tag="a_lo")

                for m in range(KD):
                    ms = slice(m * 128, (m + 1) * 128)
                    pa = psp.tile([128, NV], F32, tag="p8", name=f"pa{b}_{m}")
                    n = 0
                    for k in range(KD):
                        for lh, rh in ((wbth, vth), (wbth, vtl), (wbtl, vth)):
                            n += 1
                            nc.tensor.matmul(pa, lh[:, k, ms], rh[:, k, :],
                                             start=(n == 1), stop=(n == 3 * KD))
                    nc.scalar.copy(a_hi[:, m, :], pa)
                    nc.vector.scalar_tensor_tensor(out=a_lo[:, m, :], in0=pa,
                                                   scalar=1.0, in1=a_hi[:, m, :],
                                                   op0=ALU.mult, op1=ALU.subtract)

                c_ps = [None] * MQ
                for qm in range(MQ):
                    qs = slice(qm * 128, (qm + 1) * 128)
                    cp = psp.tile([128, NV], F32, tag="p8", name=f"c_ps{b}_{qm}")
                    c_ps[qm] = cp
                    n = 0
                    for m in range(KD):
                        for lh, rh in ((qth, a_hi), (qth, a_lo), (qtl, a_hi)):
                            n += 1
                            nc.tensor.matmul(cp, lh[:, m, qs], rh[:, m, :],
                                             start=(n == 1), stop=(n == 3 * KD))

                # previous batch's softmax/phase7 now that PE has A'+C queued
                if emit_tail_prev is not None:
                    emit_tail_prev()

                # C consumers: tanh -> c_hi f16, c32 f32, c_lo f16, fp8 copies
                c_hi = mid.tile([128, MQ, NV], F16, tag="c_hi")
                c_lo = mid.tile([128, MQ, NV], F16, tag="c_lo")
                c_h8 = mid.tile([128, MQ, NVP], F8, tag="c_h8")
                c_l8 = mid.tile([128, MQ, NVP], F8, tag="c_l8")
                for qm in range(MQ):
                    nc.scalar.activation(c_hi[:, qm, :], c_ps[qm], AF.Tanh)
                for qm in range(MQ):
                    c32 = sm.tile([128, NV], F32, tag="c32", bufs=2,
                                  name=f"c32_{b}_{qm}")
                    nc.scalar.activation(c32, c_ps[qm], AF.Tanh)
                    nc.vector.scalar_tensor_tensor(
                        out=c_lo[:, qm, :], in0=c32, scalar=1.0,
                        in1=c_hi[:, qm, :], op0=ALU.mult, op1=ALU.subtract)
                    nc.vector.tensor_copy(c_h8[:, qm, 0:NV], c_hi[:, qm, :])
                    nc.vector.tensor_scalar_mul(c_l8[:, qm, 0:NV], c_lo[:, qm, :], S_H)

                # ---- phase 3a: WqQT (psum x4096: f16 main + fp8 DR corr) ----
                wqqt_hiS = mid.tile([128, MQ, D], F16, tag="wqqt_hiS")
                wqqt_hi8 = mid.tile([128, MQ, D], F8, tag="wqqt_hi8")
                wqqt_lo8 = mid.tile([128, MQ, D], F8, tag="wqqt_lo8")
                for h in range(2):
                    hs = slice(h * 512, (h + 1) * 512)
                    for m in range(MQ):
                        ms = slice(m * 128, (m + 1) * 128)
                        p = psp.tile([128, 512], F32, tag="p8",
                                     name=f"pq{b}_{m}_{h}")
                        for k in range(KD):
                            nc.tensor.matmul(p, qth[:, k, ms], wqts[:, k, hs],
                                             start=(k == 0), stop=False)
                        for pr in range(KD // 2):
                            ks = slice(2 * pr, 2 * pr + 2)
                            nc.tensor.matmul(p, qh8[:, ks, ms], wqtl8[:, ks, hs],
                                             start=False, stop=False,
                                             perf_mode=PM.DoubleRow)
                            nc.tensor.matmul(p, ql8[:, ks, ms], wqth8[:, ks, hs],
                                             start=False,
                                             stop=(pr == KD // 2 - 1),
                                             perf_mode=PM.DoubleRow)
                        nc.scalar.mul(wqqt_hiS[:, m, hs], p, 1.0 / 16.0)
                        nc.vector.tensor_scalar_mul(wqqt_hi8[:, m, hs], p, 2.0**-12)
                        nc.vector.scalar_tensor_tensor(
                            out=wqqt_lo8[:, m, hs], in0=p, scalar=1.0 / 16.0,
                            in1=wqqt_hiS[:, m, hs], op0=ALU.mult, op1=ALU.subtract)

                # ---- phase 3b: WvVT ----
                for h in range(2):
                    hs = slice(h * 512, (h + 1) * 512)
                    for m in range(2):
                        rows = 128 if m == 0 else NV1
                        ms = slice(m * 128, m * 128 + rows)
                        p = psp.tile([128, 512], F32, tag="p8",
                                     name=f"pv{b}_{m}_{h}")
                        for k in range(KD):
                            nc.tensor.matmul(p[:rows, :], vth[:, k, ms],
                                             wvts[:, k, hs],
                                             start=(k == 0), stop=False)
                        for pr in range(KD // 2):
                            ks = slice(2 * pr, 2 * pr + 2)
                            nc.tensor.matmul(p[:rows, :], vh8[:, ks, ms],
                                             wvtl8[:, ks, hs], start=False,
                                             stop=False, perf_mode=PM.DoubleRow)
                            nc.tensor.matmul(p[:rows, :], vl8[:, ks, ms],
                                             wvth8[:, ks, hs], start=False,
                                             stop=(pr == KD // 2 - 1),
                                             perf_mode=PM.DoubleRow)
                        nc.scalar.mul(wvvt_hiS[:rows, m, hs], p[:rows, :], 1.0 / 16.0)
                        nc.vector.tensor_scalar_mul(wvvt_hi8[:rows, m, hs],
                                                    p[:rows, :], 2.0**-12)
                        nc.vector.scalar_tensor_tensor(
                            out=wvvt_lo8[:rows, m, hs], in0=p[:rows, :],
                            scalar=1.0 / 16.0, in1=wvvt_hiS[:rows, m, hs],
                            op0=ALU.mult, op1=ALU.subtract)

                # ---- CT: identity-matmul transposes of c_hi and c_lo ----
                for mv in range(2):
                    rows = 128 if mv == 0 else NV1
                    ctp = psp.tile([128, NQ], F32, tag="p8",
                                   name=f"ctp{b}_{mv}")
                    for mq in range(MQ):
                        nc.tensor.matmul(
                            ctp[:rows, mq * 128:(mq + 1) * 128],
                            c_hi[:, mq, mv * 128:mv * 128 + rows],
                            identh, start=(mq == 0), stop=(mq == MQ - 1))
                    nc.scalar.copy(ct_hi[:rows, mv, :], ctp[:rows, :])
                    nc.vector.tensor_copy(ct_h8[:rows, mv, :], ctp[:rows, :])
                    ctpl = psp.tile([128, NQ], F32, tag="p8",
                                    name=f"ctpl{b}_{mv}")
                    for mq in range(MQ):
                        nc.tensor.matmul(
                            ctpl[:rows, mq * 128:(mq + 1) * 128],
                            c_lo[:, mq, mv * 128:mv * 128 + rows],
                            identh, start=(mq == 0), stop=(mq == MQ - 1))
                    nc.vector.tensor_scalar_mul(ct_l8[:rows, mv, :],
                                                ctpl[:rows, :], S_H)

                # ---- phase 4: H_v + h_v dot ----
                hv_m_l = [None] * KD
                h_v_ps = psp.tile([1, NV], F32, tag="pD", bufs=2, name=f"hv_acc{b}")

                def emit_hv(m):
                    ms = slice(m * 128, (m + 1) * 128)
                    t2 = psp.tile([128, NV], F32, tag="p8", name=f"hv2_{b}_{m}")
                    for kq in range(MQ):
                        nc.tensor.matmul(t2, wqqt_hiS[:, kq, ms], c_hi[:, kq, :],
                                         start=(kq == 0), stop=False)
                    for pr in range(MQ // 2):
                        ks = slice(2 * pr, 2 * pr + 2)
                        nc.tensor.matmul(t2, wqqt_hi8[:, ks, ms],
                                         c_l8[:, ks, 0:NV], start=False,
                                         stop=False, perf_mode=PM.DoubleRow)
                        nc.tensor.matmul(t2, wqqt_lo8[:, ks, ms],
                                         c_h8[:, ks, 0:NV], start=False,
                                         stop=False, perf_mode=PM.DoubleRow)
                    for kv in range(2):
                        rows = 128 if kv == 0 else NV1
                        nc.tensor.matmul(t2[:, kv * 128:kv * 128 + rows],
                                         wvvt_hiS[:rows, kv, ms],
                                         identh[:rows, :rows],
                                         start=False, stop=(kv == 1))
                    hv_m = sm.tile([128, NV], F16, tag="hvm", bufs=2,
                                   name=f"hvm{b}_{m}")
                    nc.scalar.activation(hv_m, t2, AF.Tanh, scale=1.0 / S_H)
                    hv_m_l[m] = hv_m

                def emit_hv_dot(m):
                    nc.tensor.matmul(h_v_ps, whv_sb[:, m:m + 1], hv_m_l[m],
                                     start=(m == 0), stop=(m == KD - 1))

                for m in range(KD + 1):
                    if m < KD:
                        emit_hv(m)
                    if m >= 1:
                        emit_hv_dot(m - 1)

                last = (b == nb - 1)

                # ---- phases 6+7 closures (deferred emission; see below) ----
                def softmax_bcast(h_ps, n, tagp, b=b):
                    negm = sm.tile([1, 1], F32, tag=f"negm{tagp}")
                    nc.vector.reduce_max(negm, h_ps, axis=AX.X, negate=True)
                    ex = sm.tile([1, n], F16, tag=f"ex{tagp}")
                    ssum = sm.tile([1, 1], F32, tag=f"ssum{tagp}")
                    nc.scalar.activation(ex, h_ps, AF.Exp, bias=negm,
                                         accum_out=ssum)
                    rs = sm.tile([1, 1], F32, tag=f"rs{tagp}")
                    nc.vector.reciprocal(rs, ssum)
                    ones_s = sm.tile([1, 128], F16, tag=f"ones_s{tagp}")
                    nc.vector.tensor_scalar_mul(ones_s, ones_row, rs)
                    ab_ps = psp.tile([128, n], F32, tag="p8",
                                     name=f"abps{tagp}{b}")
                    nc.tensor.matmul(ab_ps, ones_s, ex, start=True, stop=True)
                    ab = sm.tile([128, n], F16, tag=f"ab{tagp}")
                    nc.scalar.copy(ab, ab_ps)
                    return ab

                def emit_tail_v(b=b, h_v_ps=h_v_ps, vth=vth):
                    av_b = softmax_bcast(h_v_ps, NV, "v", b=b)
                    vhat_sb = sm.tile([128, KD], F32, tag="vhat")
                    scrv = sm.tile([128, NV], F16, tag="scrv")
                    for k in range(KD):
                        nc.vector.scalar_tensor_tensor(
                            out=scrv, in0=vth[:, k, :], scalar=1.0, in1=av_b,
                            op0=ALU.mult, op1=ALU.mult,
                            accum_out=vhat_sb[:, k:k + 1])
                    nc.sync.dma_start(out=OV_d[b].rearrange("(k p) -> p k", p=128),
                                      in_=vhat_sb)

                # ---- phase 5: H_q + h_q dot ----
                hq_m_l = [None] * KD
                h_q_ps = psp.tile([1, NQ], F32, tag="pD", bufs=2, name=f"hq_acc{b}")

                def emit_hq(m):
                    ms = slice(m * 128, (m + 1) * 128)
                    t2 = psp.tile([128, NQ], F32, tag="p8",
                                  name=f"hq2_{b}_{m}")
                    for kv in range(2):
                        rows = 128 if kv == 0 else NV1
                        nc.tensor.matmul(t2, wvvt_hiS[:rows, kv, ms],
                                         ct_hi[:rows, kv, :],
                                         start=(kv == 0), stop=False)
                    nc.tensor.matmul(t2, wvvt_hi8[:, :, ms], ct_l8[:, :, :],
                                     start=False, stop=False,
                                     perf_mode=PM.DoubleRow)
                    nc.tensor.matmul(t2, wvvt_lo8[:, :, ms], ct_h8[:, :, :],
                                     start=False, stop=False,
                                     perf_mode=PM.DoubleRow)
                    for kq in range(MQ):
                        nc.tensor.matmul(t2[:, kq * 128:(kq + 1) * 128],
                                         wqqt_hiS[:, kq, ms], identh,
                                         start=False, stop=(kq == MQ - 1))
                    hq_m = sm.tile([128, NQ], F16, tag="hqm", bufs=2,
                                   name=f"hqm{b}_{m}")
                    nc.scalar.activation(hq_m, t2, AF.Tanh, scale=1.0 / S_H)
                    hq_m_l[m] = hq_m

                def emit_hq_dot(m):
                    nc.tensor.matmul(h_q_ps, whq_sb[:, m:m + 1], hq_m_l[m],
                                     start=(m == 0), stop=(m == KD - 1))

                # Last batch: emit v-tail mid-phase-5 so it overlaps PE work.
                for m in range(KD + 1):
                    if m < KD:
                        emit_hq(m)
                    if m >= 1:
                        emit_hq_dot(m - 1)
                    if last and m == 2:
                        emit_tail_v()

                def emit_tail_q(b=b, h_q_ps=h_q_ps, qth=qth):
                    aq_b = softmax_bcast(h_q_ps, NQ, "q", b=b)
                    qhat_sb = sm.tile([128, KD], F32, tag="qhat")
                    scrq = sm.tile([128, NQ], F16, tag="scrq")
                    for k in range(KD):
                        nc.vector.scalar_tensor_tensor(
                            out=scrq, in0=qth[:, k, :], scalar=1.0, in1=aq_b,
                            op0=ALU.mult, op1=ALU.mult,
                            accum_out=qhat_sb[:, k:k + 1])
                    nc.sync.dma_start(out=OQ_d[b].rearrange("(k p) -> p k", p=128),
                                      in_=qhat_sb)

                if b == nb - 1:
                    emit_tail_q()
                    emit_tail_prev = None
                else:
                    emit_tail_prev = lambda tv=emit_tail_v, tq=emit_tail_q: (
                        tv(), tq())

    nc.finalize()
    return nc


_BUILT = {}


def _split(x):
    hi = x.astype(np.float16)
    lo = (x - hi.astype(np.float32)).astype(np.float16)
    return np.ascontiguousarray(hi), np.ascontiguousarray(lo)


def _to8(x, scale=1.0):
    return np.ascontiguousarray(
        (np.asarray(x, np.float32) * scale).astype(ml_dtypes.float8_e4m3))


def kernel(V, Q, W_b, W_v, W_q, w_hv, w_hq, _trace=False):
    V = np.asarray(V, dtype=np.float32)
    Q = np.asarray(Q, dtype=np.float32)
    nb = B // NCORES
    QTh, QTl = _split(Q.transpose(0, 2, 1))      # [B, D, NQ] f16
    VTh, VTl = _split(V.transpose(0, 2, 1))      # [B, D, NV] f16
    WbTh, WbTl = _split(np.asarray(W_b, dtype=np.float32).T)
    WqTh, WqTl = _split(np.asarray(W_q, dtype=np.float32).T)
    WvTh, WvTl = _split(np.asarray(W_v, dtype=np.float32).T)
    WqTS = np.ascontiguousarray(WqTh.astype(np.float32) * S_W).astype(np.float16)
    WvTS = np.ascontiguousarray(WvTh.astype(np.float32) * S_W).astype(np.float16)
    whv = np.ascontiguousarray(np.asarray(w_hv, dtype=np.float32).astype(np.float16))
    whq = np.ascontiguousarray(np.asarray(w_hq, dtype=np.float32).astype(np.float16))

    if nb not in _BUILT:
        _BUILT[nb] = build(nb)
    nc = _BUILT[nb]

    shared = {
        "WbTh": WbTh, "WbTl": WbTl,
        "WqTS": WqTS, "WqTh8": _to8(WqTh), "WqTl8": _to8(WqTl, S_W),
        "WvTS": WvTS, "WvTh8": _to8(WvTh), "WvTl8": _to8(WvTl, S_W),
        "whv": whv, "whq": whq,
    }
    in_maps = []
    for c in range(NCORES):
        sl = slice(c * nb, (c + 1) * nb)
        m = {
            "QTh": np.ascontiguousarray(QTh[sl]), "QTl": np.ascontiguousarray(QTl[sl]),
            "VTh": np.ascontiguousarray(VTh[sl]), "VTl": np.ascontiguousarray(VTl[sl]),
        }
        m.update(shared)
        in_maps.append(m)

    out = run_bass_kernel_spmd(nc, in_maps, core_ids=list(range(NCORES)),
                               trace=_trace)
    v_hat = np.concatenate([out.results[c]["OV"] for c in range(NCORES)], axis=0)
    q_hat = np.concatenate([out.results[c]["OQ"] for c in range(NCORES)], axis=0)
    if _trace:
        kernel._last_exec_ns = out.exec_time_ns
        kernel._last_results = out
    return (v_hat, q_hat)


# revision 3
# speedup vs baseline: 1.0037x; 1.0009x over previous
"""CoAttention forward on 8 TRN2 NeuronCores — fp8-corrected restructured version.

Data-parallel over batch B=64 (8 batches/core). Key structure per batch b
(Q [512,1024], V [196,1024], D=1024):

  A'   = Wb @ V^T            [D, NV]   3-pass f16 (hi/lo split on chip)
  C    = tanh(Q @ A')        [NQ, NV]  3-pass f16; stored f16 hi + f16 lo
  WqQT = Q @ Wq^T            [NQ, D]   psum x4096: f16 main + fp8 DoubleRow corr
  WvVT = V @ Wv^T            [NV, D]   same
  H_v  = tanh(WqQT^T C + WvVT^T)   one psum x256: f16 main + fp8 DR corr
                                    + identity-matmul transpose additive
  H_q  = tanh(WvVT^T C^T + WqQT^T) same
  h dots in f16, softmax with on-chip PE broadcast, phase7 DVE weighted sums.

kernel(**inputs) takes FULL inputs, shards internally, returns (v_hat, q_hat).
"""
import numpy as np
import ml_dtypes

import concourse.bass as bass
import concourse.mybir as mybir
import concourse.tile as tile
from concourse import bacc
from concourse.bass_utils import run_bass_kernel_spmd
from concourse.masks import make_identity

AF = mybir.ActivationFunctionType
ALU = mybir.AluOpType
AX = mybir.AxisListType
PM = mybir.MatmulPerfMode
F32 = mybir.dt.float32
F16 = mybir.dt.float16
F8 = mybir.dt.float8e4

B, NV, NQ, D = 64, 196, 512, 1024
NCORES = 8
NB = B // NCORES          # batches per core
KD = D // 128             # 8 feature k-tiles
MQ = NQ // 128            # 4 NQ m-tiles
NV1 = NV - 128            # 68 (second NV tile)
NVP = 208                 # NV padded to /16 for fp8 DoubleRow k-tile stride

S_W = 4096.0              # weight-GEMM psum scale
S_H = 256.0               # H-path psum scale (hiS tensors are 256*value)


def build(nb=NB):
    nc = bacc.Bacc(None, target_bir_lowering=False)

    QTh_d = nc.dram_tensor("QTh", [nb, D, NQ], F16, kind="ExternalInput")
    QTl_d = nc.dram_tensor("QTl", [nb, D, NQ], F16, kind="ExternalInput")
    VTh_d = nc.dram_tensor("VTh", [nb, D, NV], F16, kind="ExternalInput")
    VTl_d = nc.dram_tensor("VTl", [nb, D, NV], F16, kind="ExternalInput")
    WbTh_d = nc.dram_tensor("WbTh", [D, D], F16, kind="ExternalInput")
    WbTl_d = nc.dram_tensor("WbTl", [D, D], F16, kind="ExternalInput")
    WqTS_d = nc.dram_tensor("WqTS", [D, D], F16, kind="ExternalInput")   # Wq^T*4096
    WqTh8_d = nc.dram_tensor("WqTh8", [D, D], F8, kind="ExternalInput")  # fp8(Wq^T)
    WqTl8_d = nc.dram_tensor("WqTl8", [D, D], F8, kind="ExternalInput")  # fp8(lo*4096)
    WvTS_d = nc.dram_tensor("WvTS", [D, D], F16, kind="ExternalInput")
    WvTh8_d = nc.dram_tensor("WvTh8", [D, D], F8, kind="ExternalInput")
    WvTl8_d = nc.dram_tensor("WvTl8", [D, D], F8, kind="ExternalInput")
    whv_d = nc.dram_tensor("whv", [D, 1], F16, kind="ExternalInput")
    whq_d = nc.dram_tensor("whq", [D, 1], F16, kind="ExternalInput")
    OV_d = nc.dram_tensor("OV", [nb, D], F32, kind="ExternalOutput")
    OQ_d = nc.dram_tensor("OQ", [nb, D], F32, kind="ExternalOutput")

    with tile.TileContext(nc) as tc:
        with (
            tc.tile_pool(name="wsb", bufs=1) as wsb,
            tc.tile_pool(name="iop", bufs=2) as iop,
            tc.tile_pool(name="io8", bufs=1) as io8,
            tc.tile_pool(name="mid", bufs=1) as mid,
            tc.tile_pool(name="sm", bufs=1) as sm,
            tc.tile_pool(name="psp", bufs=6, space="PSUM") as psp,
        ):
            # ---- persistent weights (per-k chunked DMAs for fine-grained deps
            # so the first A'/C/WqQT matmuls start as soon as their chunk lands) ----
            def wtile(name, src, dt=F16):
                t = wsb.tile([128, KD, D], dt, name=name)
                r = src.rearrange("(k p) d -> p k d", p=128)
                for k in range(KD):
                    nc.sync.dma_start(out=t[:, k, :], in_=r[:, k, :])
                return t

            wbth = wtile("wbth", WbTh_d)
            wbtl = wtile("wbtl", WbTl_d)

            def load_inputs(b):
                vth = iop.tile([128, KD, NV], F16, tag="vth", name=f"vth{b}")
                nc.sync.dma_start(out=vth, in_=VTh_d[b].rearrange("(k p) n -> p k n", p=128))
                vtl = iop.tile([128, KD, NV], F16, tag="vtl", name=f"vtl{b}")
                nc.sync.dma_start(out=vtl, in_=VTl_d[b].rearrange("(k p) n -> p k n", p=128))
                qth = iop.tile([128, KD, NQ], F16, tag="qth", name=f"qth{b}")
                nc.sync.dma_start(out=qth, in_=QTh_d[b].rearrange("(k p) n -> p k n", p=128))
                qtl = iop.tile([128, KD, NQ], F16, tag="qtl", name=f"qtl{b}")
                nc.sync.dma_start(out=qtl, in_=QTl_d[b].rearrange("(k p) n -> p k n", p=128))
                qh8 = io8.tile([128, KD, NQ], F8, tag="qh8", name=f"qh8{b}")
                nc.vector.tensor_copy(qh8, qth)
                ql8 = io8.tile([128, KD, NQ], F8, tag="ql8", name=f"ql8{b}")
                nc.vector.tensor_scalar_mul(ql8, qtl, S_W)
                vh8 = io8.tile([128, KD, NVP], F8, tag="vh8", name=f"vh8{b}")
                nc.vector.tensor_copy(vh8[:, :, 0:NV], vth)
                vl8 = io8.tile([128, KD, NVP], F8, tag="vl8", name=f"vl8{b}")
                nc.vector.tensor_scalar_mul(vl8[:, :, 0:NV], vtl, S_W)
                return qth, qtl, vth, vtl, qh8, ql8, vh8, vl8

            ins0 = load_inputs(0)

            wqts = wtile("wqts", WqTS_d)
            wqth8 = wtile("wqth8", WqTh8_d, F8)
            wqtl8 = wtile("wqtl8", WqTl8_d, F8)
            wvts = wtile("wvts", WvTS_d)
            wvth8 = wtile("wvth8", WvTh8_d, F8)
            wvtl8 = wtile("wvtl8", WvTl8_d, F8)
            whv_sb = wsb.tile([128, KD], F16)
            nc.sync.dma_start(out=whv_sb, in_=whv_d[:, 0].rearrange("(k p) -> p k", p=128))
            whq_sb = wsb.tile([128, KD], F16)
            nc.sync.dma_start(out=whq_sb, in_=whq_d[:, 0].rearrange("(k p) -> p k", p=128))
            identh = wsb.tile([128, 128], F16)
            make_identity(nc, identh)
            ones_row = wsb.tile([1, 128], F32)
            nc.vector.memset(ones_row, 1.0)

            # per-batch tensors with static zero rows for fp8 DR padding:
            # allocated once; rows [NV1:128, 1, :] memset once, never rewritten.
            ct_hi = wsb.tile([128, 2, NQ], F16)
            ct_h8 = wsb.tile([128, 2, NQ], F8)
            ct_l8 = wsb.tile([128, 2, NQ], F8)
            wvvt_hiS = wsb.tile([128, 2, D], F16)
            wvvt_hi8 = wsb.tile([128, 2, D], F8)
            wvvt_lo8 = wsb.tile([128, 2, D], F8)
            # engine base partition must be 0/32/64/96: zero [64:128] once; the
            # per-batch writes re-fill rows [64:NV1] so only [NV1:128] stay 0.
            for t in (ct_h8, ct_l8, wvvt_hi8, wvvt_lo8):
                nc.vector.memset(t[64:128, 1, :], 0.0)

            emit_tail_prev = None
            for b in range(nb):
                qth, qtl, vth, vtl, qh8, ql8, vh8, vl8 = (
                    ins0 if b == 0 else load_inputs(b))

                # ---- phase 1: A' (3-pass); phase 2: C (3-pass, per q-tile) ----
                a_hi = mid.tile([128, KD, NV], F16, tag="a_hi")
                a_lo = mid.tile([128, KD, NV], F16, tag="a_lo")

                for m in range(KD):
                    ms = slice(m * 128, (m + 1) * 128)
                    pa = psp.tile([128, NV], F32, tag="p8", name=f"pa{b}_{m}")
                    n = 0
                    for k in range(KD):
                        for lh, rh in ((wbth, vth), (wbth, vtl), (wbtl, vth)):
                            n += 1
                            nc.tensor.matmul(pa, lh[:, k, ms], rh[:, k, :],
                                             start=(n == 1), stop=(n == 3 * KD))
                    nc.scalar.copy(a_hi[:, m, :], pa)
                    nc.vector.scalar_tensor_tensor(out=a_lo[:, m, :], in0=pa,
                                                   scalar=1.0, in1=a_hi[:, m, :],
                                                   op0=ALU.mult, op1=ALU.subtract)

                c_ps = [None] * MQ
                for qm in range(MQ):
                    qs = slice(qm * 128, (qm + 1) * 128)
                    cp = psp.tile([128, NV], F32, tag="p8", name=f"c_ps{b}_{qm}")
                    c_ps[qm] = cp
                    n = 0
                    for m in range(KD):
                        for lh, rh in ((qth, a_hi), (qth, a_lo), (qtl, a_hi)):
                            n += 1
                            nc.tensor.matmul(cp, lh[:, m, qs], rh[:, m, :],
                                             start=(n == 1), stop=(n == 3 * KD))

                # previous batch's softmax/phase7 now that PE has A'+C queued
                if emit_tail_prev is not None:
                    emit_tail_prev()

                # C consumers: tanh -> c_hi f16, c32 f32, c_lo f16, fp8 copies
                c_hi = mid.tile([128, MQ, NV], F16, tag="c_hi")
                c_lo = mid.tile([128, MQ, NV], F16, tag="c_lo")
                c_h8 = mid.tile([128, MQ, NVP], F8, tag="c_h8")
                c_l8 = mid.tile([128, MQ, NVP], F8, tag="c_l8")
                for qm in range(MQ):
                    nc.scalar.activation(c_hi[:, qm, :], c_ps[qm], AF.Tanh)
                for qm in range(MQ):
                    c32 = sm.tile([128, NV], F32, tag="c32", bufs=2,
                                  name=f"c32_{b}_{qm}")
                    nc.scalar.activation(c32, c_ps[qm], AF.Tanh)
                    nc.vector.scalar_tensor_tensor(
                        out=c_lo[:, qm, :], in0=c32, scalar=1.0,
                        in1=c_hi[:, qm, :], op0=ALU.mult, op1=ALU.subtract)
                    nc.vector.tensor_copy(c_h8[:, qm, 0:NV], c_hi[:, qm, :])
                    nc.vector.tensor_scalar_mul(c_l8[:, qm, 0:NV], c_lo[:, qm, :], S_H)

                # ---- phase 3a: WqQT (psum x4096: f16 main + fp8 DR corr) ----
                wqqt_hiS = mid.tile([128, MQ, D], F16, tag="wqqt_hiS")
                wqqt_hi8 = mid.tile([128, MQ, D], F8, tag="wqqt_hi8")
                wqqt_lo8 = mid.tile([128, MQ, D], F8, tag="wqqt_lo8")
                for h in range(2):
                    hs = slice(h * 512, (h + 1) * 512)
                    for m in range(MQ):
                        ms = slice(m * 128, (m + 1) * 128)
                        p = psp.tile([128, 512], F32, tag="p8",
                                     name=f"pq{b}_{m}_{h}")
                        for k in range(KD):
                            nc.tensor.matmul(p, qth[:, k, ms], wqts[:, k, hs],
                                             start=(k == 0), stop=False)
                        for pr in range(KD // 2):
                            ks = slice(2 * pr, 2 * pr + 2)
                            nc.tensor.matmul(p, qh8[:, ks, ms], wqtl8[:, ks, hs],
                                             start=False, stop=False,
                                             perf_mode=PM.DoubleRow)
                            nc.tensor.matmul(p, ql8[:, ks, ms], wqth8[:, ks, hs],
                                             start=False,
                                             stop=(pr == KD // 2 - 1),
                                             perf_mode=PM.DoubleRow)
                        nc.scalar.mul(wqqt_hiS[:, m, hs], p, 1.0 / 16.0)
                        nc.vector.tensor_scalar_mul(wqqt_hi8[:, m, hs], p, 2.0**-12)
                        nc.vector.scalar_tensor_tensor(
                            out=wqqt_lo8[:, m, hs], in0=p, scalar=1.0 / 16.0,
                            in1=wqqt_hiS[:, m, hs], op0=ALU.mult, op1=ALU.subtract)

                # ---- phase 3b: WvVT ----
                for h in range(2):
                    hs = slice(h * 512, (h + 1) * 512)
                    for m in range(2):
                        rows = 128 if m == 0 else NV1
                        ms = slice(m * 128, m * 128 + rows)
                        p = psp.tile([128, 512], F32, tag="p8",
                                     name=f"pv{b}_{m}_{h}")
                        for k in range(KD):
                            nc.tensor.matmul(p[:rows, :], vth[:, k, ms],
                                             wvts[:, k, hs],
                                             start=(k == 0), stop=False)
                        for pr in range(KD // 2):
                            ks = slice(2 * pr, 2 * pr + 2)
                            nc.tensor.matmul(p[:rows, :], vh8[:, ks, ms],
                                             wvtl8[:, ks, hs], start=False,
                                             stop=False, perf_mode=PM.DoubleRow)
                            nc.tensor.matmul(p[:rows, :], vl8[:, ks, ms],
                                             wvth8[:, ks, hs], start=False,
                                             stop=(pr == KD // 2 - 1),
                                             perf_mode=PM.DoubleRow)
                        nc.scalar.mul(wvvt_hiS[:rows, m, hs], p[:rows, :], 1.0 / 16.0)
                        nc.vector.tensor_scalar_mul(wvvt_hi8[:rows, m, hs],
                                                    p[:rows, :], 2.0**-12)
                        nc.vector.scalar_tensor_tensor(
                            out=wvvt_lo8[:rows, m, hs], in0=p[:rows, :],
                            scalar=1.0 / 16.0, in1=wvvt_hiS[:rows, m, hs],
                            op0=ALU.mult, op1=ALU.subtract)

                # ---- CT: identity-matmul transposes of c_hi and c_lo ----
                for mv in range(2):
                    rows = 128 if mv == 0 else NV1
                    ctp = psp.tile([128, NQ], F32, tag="p8",
                                   name=f"ctp{b}_{mv}")
                    for mq in range(MQ):
                        nc.tensor.matmul(
                            ctp[:rows, mq * 128:(mq + 1) * 128],
                            c_hi[:, mq, mv * 128:mv * 128 + rows],
                            identh, start=(mq == 0), stop=(mq == MQ - 1))
                    nc.scalar.copy(ct_hi[:rows, mv, :], ctp[:rows, :])
                    nc.vector.tensor_copy(ct_h8[:rows, mv, :], ctp[:rows, :])
                    ctpl = psp.tile([128, NQ], F32, tag="p8",
                                    name=f"ctpl{b}_{mv}")
                    for mq in range(MQ):
                        nc.tensor.matmul(
                            ctpl[:rows, mq * 128:(mq + 1) * 128],
                            c_lo[:, mq, mv * 128:mv * 128 + rows],
                            identh, start=(mq == 0), stop=(mq == MQ - 1))
                    nc.vector.tensor_scalar_mul(ct_l8[:rows, mv, :],
                                                ctpl[:rows, :], S_H)

                # ---- phase 4: H_v + h_v dot ----
                hv_m_l = [None] * KD
                hv_ps_box = [None]

                def emit_hv(m):
                    ms = slice(m * 128, (m + 1) * 128)
                    t2 = psp.tile([128, NV], F32, tag="p8", name=f"hv2_{b}_{m}")
                    for kq in range(MQ):
                        nc.tensor.matmul(t2, wqqt_hiS[:, kq, ms], c_hi[:, kq, :],
                                         start=(kq == 0), stop=False)
                    for pr in range(MQ // 2):
                        ks = slice(2 * pr, 2 * pr + 2)
                        nc.tensor.matmul(t2, wqqt_hi8[:, ks, ms],
                                         c_l8[:, ks, 0:NV], start=False,
                                         stop=False, perf_mode=PM.DoubleRow)
                        nc.tensor.matmul(t2, wqqt_lo8[:, ks, ms],
                                         c_h8[:, ks, 0:NV], start=False,
                                         stop=False, perf_mode=PM.DoubleRow)
                    for kv in range(2):
                        rows = 128 if kv == 0 else NV1
                        nc.tensor.matmul(t2[:, kv * 128:kv * 128 + rows],
                                         wvvt_hiS[:rows, kv, ms],
                                         identh[:rows, :rows],
                                         start=False, stop=(kv == 1))
                    hv_m = sm.tile([128, NV], F16, tag="hvm", bufs=2,
                                   name=f"hvm{b}_{m}")
                    nc.scalar.activation(hv_m, t2, AF.Tanh, scale=1.0 / S_H)
                    hv_m_l[m] = hv_m

                def emit_hv_dot(m):
                    nc.tensor.matmul(hv_ps_box[0], whv_sb[:, m:m + 1], hv_m_l[m],
                                     start=(m == 0), stop=(m == KD - 1))

                def emit_phase_hv(b=b, mid_hook=None):
                    hv_ps_box[0] = psp.tile([1, NV], F32, tag="pD", bufs=2,
                                            name=f"hv_acc{b}")
                    for m in range(KD + 1):
                        if m < KD:
                            emit_hv(m)
                        if m >= 1:
                            emit_hv_dot(m - 1)
                        if mid_hook is not None and m == 2:
                            mid_hook()
                    return hv_ps_box[0]

                last = (b == nb - 1)

                # ---- phases 6+7 closures (deferred emission; see below) ----
                def softmax_bcast(h_ps, n, tagp, b=b):
                    negm = sm.tile([1, 1], F32, tag=f"negm{tagp}")
                    nc.vector.reduce_max(negm, h_ps, axis=AX.X, negate=True)
                    ex = sm.tile([1, n], F16, tag=f"ex{tagp}")
                    ssum = sm.tile([1, 1], F32, tag=f"ssum{tagp}")
                    nc.scalar.activation(ex, h_ps, AF.Exp, bias=negm,
                                         accum_out=ssum)
                    rs = sm.tile([1, 1], F32, tag=f"rs{tagp}")
                    nc.vector.reciprocal(rs, ssum)
                    ones_s = sm.tile([1, 128], F16, tag=f"ones_s{tagp}")
                    nc.vector.tensor_scalar_mul(ones_s, ones_row, rs)
                    ab_ps = psp.tile([128, n], F32, tag="p8",
                                     name=f"abps{tagp}{b}")
                    nc.tensor.matmul(ab_ps, ones_s, ex, start=True, stop=True)
                    ab = sm.tile([128, n], F16, tag=f"ab{tagp}")
                    nc.scalar.copy(ab, ab_ps)
                    return ab

                def emit_tail_v(b=b, vth=vth):
                    h_v_ps = hv_ps_box[0]
                    av_b = softmax_bcast(h_v_ps, NV, "v", b=b)
                    vhat_sb = sm.tile([128, KD], F32, tag="vhat")
                    scrv = sm.tile([128, NV], F16, tag="scrv")
                    for k in range(KD):
                        nc.vector.scalar_tensor_tensor(
                            out=scrv, in0=vth[:, k, :], scalar=1.0, in1=av_b,
                            op0=ALU.mult, op1=ALU.mult,
                            accum_out=vhat_sb[:, k:k + 1])
                    nc.sync.dma_start(out=OV_d[b].rearrange("(k p) -> p k", p=128),
                                      in_=vhat_sb)

                # ---- phase 5: H_q + h_q dot ----
                hq_m_l = [None] * KD
                hq_ps_box = [None]

                def emit_hq(m):
                    ms = slice(m * 128, (m + 1) * 128)
                    t2 = psp.tile([128, NQ], F32, tag="p8",
                                  name=f"hq2_{b}_{m}")
                    for kv in range(2):
                        rows = 128 if kv == 0 else NV1
                        nc.tensor.matmul(t2, wvvt_hiS[:rows, kv, ms],
                                         ct_hi[:rows, kv, :],
                                         start=(kv == 0), stop=False)
                    nc.tensor.matmul(t2, wvvt_hi8[:, :, ms], ct_l8[:, :, :],
                                     start=False, stop=False,
                                     perf_mode=PM.DoubleRow)
                    nc.tensor.matmul(t2, wvvt_lo8[:, :, ms], ct_h8[:, :, :],
                                     start=False, stop=False,
                                     perf_mode=PM.DoubleRow)
                    for kq in range(MQ):
                        nc.tensor.matmul(t2[:, kq * 128:(kq + 1) * 128],
                                         wqqt_hiS[:, kq, ms], identh,
                                         start=False, stop=(kq == MQ - 1))
                    hq_m = sm.tile([128, NQ], F16, tag="hqm", bufs=2,
                                   name=f"hqm{b}_{m}")
                    nc.scalar.activation(hq_m, t2, AF.Tanh, scale=1.0 / S_H)
                    hq_m_l[m] = hq_m

                def emit_hq_dot(m):
                    nc.tensor.matmul(hq_ps_box[0], whq_sb[:, m:m + 1], hq_m_l[m],
                                     start=(m == 0), stop=(m == KD - 1))

                def emit_phase_hq(b=b, mid_hook=None):
                    hq_ps_box[0] = psp.tile([1, NQ], F32, tag="pD", bufs=2,
                                            name=f"hq_acc{b}")
                    for m in range(KD + 1):
                        if m < KD:
                            emit_hq(m)
                        if m >= 1:
                            emit_hq_dot(m - 1)
                        if mid_hook is not None and m == 2:
                            mid_hook()
                    return hq_ps_box[0]

                def emit_tail_q(b=b, qth=qth):
                    h_q_ps = hq_ps_box[0]
                    aq_b = softmax_bcast(h_q_ps, NQ, "q", b=b)
                    qhat_sb = sm.tile([128, KD], F32, tag="qhat")
                    scrq = sm.tile([128, NQ], F16, tag="scrq")
                    for k in range(KD):
                        nc.vector.scalar_tensor_tensor(
                            out=scrq, in0=qth[:, k, :], scalar=1.0, in1=aq_b,
                            op0=ALU.mult, op1=ALU.mult,
                            accum_out=qhat_sb[:, k:k + 1])
                    nc.sync.dma_start(out=OQ_d[b].rearrange("(k p) -> p k", p=128),
                                      in_=qhat_sb)

                if last:
                    # v-tail emitted mid-H_q so it overlaps PE work.
                    emit_phase_hv()
                    emit_phase_hq(mid_hook=emit_tail_v)
                    emit_tail_q()
                    emit_tail_prev = None
                else:
                    emit_phase_hv()
                    emit_phase_hq()
                    emit_tail_prev = lambda tv=emit_tail_v, tq=emit_tail_q: (
                        tv(), tq())

    nc.finalize()
    return nc


_BUILT = {}


def _split(x):
    hi = x.astype(np.float16)
    lo = (x - hi.astype(np.float32)).astype(np.float16)
    return np.ascontiguousarray(hi), np.ascontiguousarray(lo)


def _to8(x, scale=1.0):
    return np.ascontiguousarray(
        (np.asarray(x, np.float32) * scale).astype(ml_dtypes.float8_e4m3))


def kernel(V, Q, W_b, W_v, W_q, w_hv, w_hq, _trace=False):
    V = np.asarray(V, dtype=np.float32)
    Q = np.asarray(Q, dtype=np.float32)
    nb = B // NCORES
    QTh, QTl = _split(Q.transpose(0, 2, 1))      # [B, D, NQ] f16
    VTh, VTl = _split(V.transpose(0, 2, 1))      # [B, D, NV] f16
    WbTh, WbTl = _split(np.asarray(W_b, dtype=np.float32).T)
    WqTh, WqTl = _split(np.asarray(W_q, dtype=np.float32).T)
    WvTh, WvTl = _split(np.asarray(W_v, dtype=np.float32).T)
    WqTS = np.ascontiguousarray(WqTh.astype(np.float32) * S_W).astype(np.float16)
    WvTS = np.ascontiguousarray(WvTh.astype(np.float32) * S_W).astype(np.float16)
    whv = np.ascontiguousarray(np.asarray(w_hv, dtype=np.float32).astype(np.float16))
    whq = np.ascontiguousarray(np.asarray(w_hq, dtype=np.float32).astype(np.float16))

    if nb not in _BUILT:
        _BUILT[nb] = build(nb)
    nc = _BUILT[nb]

    shared = {
        "WbTh": WbTh, "WbTl": WbTl,
        "WqTS": WqTS, "WqTh8": _to8(WqTh), "WqTl8": _to8(WqTl, S_W),
        "WvTS": WvTS, "WvTh8": _to8(WvTh), "WvTl8": _to8(WvTl, S_W),
        "whv": whv, "whq": whq,
    }
    in_maps = []
    for c in range(NCORES):
        sl = slice(c * nb, (c + 1) * nb)
        m = {
            "QTh": np.ascontiguousarray(QTh[sl]), "QTl": np.ascontiguousarray(QTl[sl]),
            "VTh": np.ascontiguousarray(VTh[sl]), "VTl": np.ascontiguousarray(VTl[sl]),
        }
        m.update(shared)
        in_maps.append(m)

    out = run_bass_kernel_spmd(nc, in_maps, core_ids=list(range(NCORES)),
                               trace=_trace)
    v_hat = np.concatenate([out.results[c]["OV"] for c in range(NCORES)], axis=0)
    q_hat = np.concatenate([out.results[c]["OQ"] for c in range(NCORES)], axis=0)
    if _trace:
        kernel._last_exec_ns = out.exec_time_ns
        kernel._last_results = out
    return (v_hat, q_hat)
